# revision 13
# baseline (speedup 1.0000x reference)
"""MoE LoRA linear layer kernel for Trainium2, data-parallel over 8 NeuronCores.

Math (per token n):
    down = h @ down_w.T                      [N, 64]
    mask[n, r] = val[n, k] if idx[n, k] == r else 0   (indices distinct per row)
    out = (down * mask) @ up_w.T             [N, 4096]

Sharding: tokens split 8 ways (2048/core); LoRA weights replicated.

Strategy (v4): device does two matmul passes + one fused DVE multiply;
all layout work happens in the host packer, all traffic is bf16
(accumulation in f32 PSUM). DMA roofline ~34 MiB/core @ ~320 GB/s.

  * h is pre-transposed AND ki-pair-packed on the host
    (ht2[pr*128+p, j*2048+n] = h[n, (2pr+j)*128+p]) so each of the 16
    loads is 1 MiB of contiguous 8 KiB descriptors.
  * down-proj: even ki chunks write PSUM partitions 0-63, odd ki
    chunks partitions 64-127 (128x64 column-tiled array mode, two
    concurrent tile streams) into ONE [128, 2048] 4-bank accumulator.
  * up-proj contracts K=128 against host-duplicated up weights
    (upw2 = [upT; upT]): the even/odd partial sums combine inside the
    matmul -- full 128x128 array, FWL weight loads.
  * the top-k scatter mask is a dense host-built maskT (bf16,
    replicated to 128 partitions); masking fuses with the PSUM->SBUF
    eviction on the DVE.
  * up-proj emits outT (stationary weights, transposed output),
    oc-pair-packed to make 1 MiB stores; the host unpacks + transposes
    while gathering the 8 shards.
"""

import sys

for p in ("/opt/trn_rl_repo", "/opt/pypackages"):
    if p not in sys.path:
        sys.path.insert(0, p)

import ml_dtypes
import numpy as np

BF16 = ml_dtypes.bfloat16

N, D_IN, D_OUT, RANK, TOPK = 16384, 4096, 4096, 64, 8
NCORES = 8
NT = N // NCORES          # tokens per core = 2048
P = 128                   # partitions
NKC = D_IN // P           # 32 contraction chunks for the down proj
NPAIR = NKC // 2          # 16 even/odd chunk pairs
QW = 512                  # matmul free width (one PSUM bank of f32)
NQ = NT // QW             # 4 free-dim tiles
NOCP = D_OUT // (2 * P)   # 16 output-row-chunk pairs for the up proj

_CACHE = {}


def _build_program():
    import concourse.bacc as bacc
    import concourse.mybir as mybir
    from concourse import tile

    f32 = mybir.dt.float32
    bf16 = mybir.dt.bfloat16
    nc = bacc.Bacc()

    ht4 = nc.declare_dram_parameter("ht4", [D_IN // 4, 4 * NT], bf16, isOutput=False)
    dwt = nc.declare_dram_parameter("dwt", [P, NKC * RANK], bf16, isOutput=False)
    upw2 = nc.declare_dram_parameter("upw2", [P, D_OUT], bf16, isOutput=False)
    maskt = nc.declare_dram_parameter("maskt", [P, NT], bf16, isOutput=False)
    outt2 = nc.declare_dram_parameter("outt2", [D_OUT // 2, 2 * NT], bf16, isOutput=True)

    with tile.TileContext(nc) as tc:
        with (
            tc.tile_pool(name="const", bufs=1) as const,
            tc.tile_pool(name="hch", bufs=3) as hch_pool,
            tc.tile_pool(name="res", bufs=1) as res_pool,
            tc.tile_pool(name="outsb", bufs=3) as out_pool,
        ):
            dwt_sb = const.tile([P, NKC * RANK], bf16, name="dwt_sb")
            upw2_sb = const.tile([P, D_OUT], bf16, name="upw2_sb")
            maskt_sb = const.tile([P, NT], bf16, name="maskt_sb")
            nc.sync.dma_start(out=dwt_sb[:], in_=dwt[:, :])

            resT = res_pool.tile([P, NT], bf16, name="resT")

            with tc.tile_pool(name="psum_dn", bufs=1, space="PSUM") as psum_dn_pool:
                # single 4-bank accumulator: partitions 0-63 = even ki
                # chunks, 64-127 = odd ki chunks
                dn = psum_dn_pool.tile([P, NT], f32, name="dn")

                NQUAD = NKC // 4
                for qr in range(NQUAD):
                    hc = hch_pool.tile([P, 4 * NT], bf16, name="hc")
                    if qr == 0:
                        # split the first load so the first matmul waits on
                        # 512 KiB, not 2 MiB (pipeline fill)
                        for j4 in range(4):
                            nc.sync.dma_start(
                                out=hc[:, j4 * NT:(j4 + 1) * NT],
                                in_=ht4[qr * P:(qr + 1) * P,
                                        j4 * NT:(j4 + 1) * NT],
                            )
                    else:
                        nc.sync.dma_start(
                            out=hc[:], in_=ht4[qr * P:(qr + 1) * P, :]
                        )
                    if qr == 1:
                        # needed only from the mask/up phase (~55 us in):
                        # slot the loads behind the first chunk quads
                        nc.sync.dma_start(out=upw2_sb[:], in_=upw2[:, :])
                        nc.sync.dma_start(out=maskt_sb[:], in_=maskt[:, :])
                    for q in range(NQ):
                        for j4 in range(4):
                            ki = 4 * qr + j4
                            j = ki % 2
                            # even ki -> psum partitions 0-63 (array cols
                            # 0-63), odd -> 64-127; the two column-tile
                            # streams run concurrently on the PE
                            nc.tensor.matmul(
                                dn[j * RANK:(j + 1) * RANK,
                                   q * QW:(q + 1) * QW],
                                lhsT=dwt_sb[:, ki * RANK:(ki + 1) * RANK],
                                rhs=hc[:, j4 * NT + q * QW:j4 * NT + (q + 1) * QW],
                                start=(qr == 0 and j4 < 2),
                                stop=(qr == NQUAD - 1 and j4 >= 2),
                                skip_group_check=True,
                            )

                # evict downT psum -> sbuf bf16 fused with the top-k mask;
                # quarter granularity so the up phase starts after q0
                for q in range(NQ):
                    cols = slice(q * QW, (q + 1) * QW)
                    nc.vector.tensor_mul(
                        resT[:, cols], maskt_sb[:, cols], dn[:, cols]
                    )

            # up-proj: outT[oc] = upw2.T @ resT with K=128 (the stacked
            # even/odd partials sum inside the contraction)
            with tc.tile_pool(name="psum_up", bufs=4, space="PSUM") as psum_up_pool:
                for ocp in range(NOCP):
                    osb = out_pool.tile([P, 2 * NT], bf16, name="osb")
                    for oc_in in range(2):
                        oc = 2 * ocp + oc_in
                        for qq in range(2):
                            pu = psum_up_pool.tile([P, 2 * QW], f32, name="pu")
                            for k in range(2):
                                q = 2 * qq + k
                                nc.tensor.matmul(
                                    pu[:, k * QW:(k + 1) * QW],
                                    lhsT=upw2_sb[:, oc * P:(oc + 1) * P],
                                    rhs=resT[:, q * QW:(q + 1) * QW],
                                    start=True,
                                    stop=True,
                                )
                            dst = osb[:, oc_in * NT + qq * 2 * QW:
                                      oc_in * NT + (qq + 1) * 2 * QW]
                            if (oc_in + qq) % 2 == 0:
                                nc.scalar.copy(out=dst, in_=pu[:])
                            else:
                                nc.vector.tensor_copy(out=dst, in_=pu[:])
                    if ocp == NOCP - 1:
                        # split the last store so the final DMA drain on the
                        # critical tail is ~256 KiB, not 1 MiB
                        w = NT // 2
                        for k in range(4):
                            nc.sync.dma_start(
                                out=outt2[ocp * P:(ocp + 1) * P,
                                          k * w:(k + 1) * w],
                                in_=osb[:, k * w:(k + 1) * w],
                            )
                    else:
                        nc.sync.dma_start(
                            out=outt2[ocp * P:(ocp + 1) * P, :], in_=osb[:]
                        )

    nc.finalize()
    return nc


def _get_program():
    if "nc" not in _CACHE:
        _CACHE["nc"] = _build_program()
    return _CACHE["nc"]


def prepare_in_maps(hidden_states, down_w, up_w, top_k_values, top_k_indices):
    h = np.ascontiguousarray(hidden_states, dtype=np.float32).astype(BF16)
    dw = np.ascontiguousarray(down_w, dtype=np.float32).astype(BF16)
    uw = np.ascontiguousarray(up_w, dtype=np.float32).astype(BF16)
    vals = np.ascontiguousarray(top_k_values, dtype=np.float32)
    idx = np.asarray(top_k_indices).astype(np.int64)

    # dwt[i, ki*64 + r] = dw[r, ki*128 + i]
    dwt = np.ascontiguousarray(
        dw.reshape(RANK, NKC, P).transpose(2, 1, 0).reshape(P, NKC * RANK)
    )
    # up weights transposed and stacked twice: K=128 contraction sums the
    # even-ki (partitions 0-63) and odd-ki (64-127) down partials
    upw2 = np.ascontiguousarray(np.vstack([uw.T, uw.T]))  # [128, 4096]

    rows = np.arange(NT)[:, None]
    in_maps = []
    for c in range(NCORES):
        s = slice(c * NT, (c + 1) * NT)
        # ht4[qr*128+p, j4*2048+n] = h[s][n, (4qr+j4)*128+p]
        ht = h[s].T  # [4096, 2048]
        ht4 = np.ascontiguousarray(
            ht.reshape(NKC // 4, 4, P, NT).transpose(0, 2, 1, 3).reshape(D_IN // 4, 4 * NT)
        )
        m = np.zeros((NT, RANK), dtype=np.float32)
        m[rows, idx[s]] = vals[s]
        mt = m.T.astype(BF16)  # [64, 2048]
        in_maps.append(
            {
                "ht4": ht4,
                "dwt": dwt,
                "upw2": upw2,
                "maskt": np.ascontiguousarray(np.vstack([mt, mt])),  # [128, 2048]
            }
        )
    return in_maps


def gather_output(results):
    # each core returns outt2 [2048, 4096] bf16 with
    # outt2[ocp*128+p, oc_in*2048+n] = outT[(2*ocp+oc_in)*128+p, n];
    # unpack to outT [4096, 2048], transpose, upcast
    outs = []
    for r in results:
        o2 = np.asarray(r["outt2"])
        outT = (
            o2.reshape(NOCP, P, 2, NT)
            .transpose(0, 2, 1, 3)
            .reshape(D_OUT, NT)
        )
        outs.append(outT.T.astype(np.float32))
    return np.concatenate(outs, axis=0)


def kernel(hidden_states, down_w, up_w, top_k_values, top_k_indices, **_kw):
    from concourse.bass_utils import run_bass_kernel_spmd

    nc = _get_program()
    in_maps = prepare_in_maps(
        hidden_states, down_w, up_w, top_k_values, top_k_indices
    )
    res = run_bass_kernel_spmd(nc, in_maps, core_ids=list(range(NCORES)))
    return gather_output(res.results)


# revision 16
# speedup vs baseline: 1.0508x; 1.0508x over previous
"""MoE LoRA linear layer kernel for Trainium2, data-parallel over 8 NeuronCores.

Math (per token n):
    down = h @ down_w.T                      [N, 64]
    mask[n, r] = val[n, k] if idx[n, k] == r else 0   (indices distinct per row)
    out = (down * mask) @ up_w.T             [N, 4096]

Sharding: tokens split 8 ways (2048/core); LoRA weights replicated.

Strategy (v4): device does two matmul passes + one fused DVE multiply;
all layout work happens in the host packer, all traffic is bf16
(accumulation in f32 PSUM). DMA roofline ~34 MiB/core @ ~320 GB/s.

  * h is pre-transposed AND ki-pair-packed on the host
    (ht2[pr*128+p, j*2048+n] = h[n, (2pr+j)*128+p]) so each of the 16
    loads is 1 MiB of contiguous 8 KiB descriptors.
  * down-proj: even ki chunks write PSUM partitions 0-63, odd ki
    chunks partitions 64-127 (128x64 column-tiled array mode, two
    concurrent tile streams) into ONE [128, 2048] 4-bank accumulator.
  * up-proj contracts K=128 against host-duplicated up weights
    (upw2 = [upT; upT]): the even/odd partial sums combine inside the
    matmul -- full 128x128 array, FWL weight loads.
  * the top-k scatter mask is a dense host-built maskT (bf16,
    replicated to 128 partitions); masking fuses with the PSUM->SBUF
    eviction on the DVE.
  * up-proj emits outT (stationary weights, transposed output),
    oc-pair-packed to make 1 MiB stores; the host unpacks + transposes
    while gathering the 8 shards.
"""

import sys

for p in ("/opt/trn_rl_repo", "/opt/pypackages"):
    if p not in sys.path:
        sys.path.insert(0, p)

import ml_dtypes
import numpy as np

BF16 = ml_dtypes.bfloat16

N, D_IN, D_OUT, RANK, TOPK = 16384, 4096, 4096, 64, 8
NCORES = 8
NT = N // NCORES          # tokens per core = 2048
P = 128                   # partitions
NKC = D_IN // P           # 32 contraction chunks for the down proj
NPAIR = NKC // 2          # 16 even/odd chunk pairs
QW = 512                  # matmul free width (one PSUM bank of f32)
NQ = NT // QW             # 4 free-dim tiles
NOCP = D_OUT // (2 * P)   # 16 output-row-chunk pairs for the up proj

_CACHE = {}


def _build_program():
    import concourse.bacc as bacc
    import concourse.mybir as mybir
    from concourse import tile

    f32 = mybir.dt.float32
    bf16 = mybir.dt.bfloat16
    nc = bacc.Bacc()

    ht4 = nc.declare_dram_parameter("ht4", [D_IN // 4, 4 * NT], bf16, isOutput=False)
    dwt = nc.declare_dram_parameter("dwt", [P, NKC * RANK], bf16, isOutput=False)
    upw2 = nc.declare_dram_parameter("upw2", [P, D_OUT], bf16, isOutput=False)
    maskt = nc.declare_dram_parameter("maskt", [P, NT], bf16, isOutput=False)
    outt2 = nc.declare_dram_parameter("outt2", [D_OUT // 2, 2 * NT], bf16, isOutput=True)

    with tile.TileContext(nc) as tc:
        with (
            tc.tile_pool(name="const", bufs=1) as const,
            tc.tile_pool(name="hch", bufs=4) as hch_pool,
            tc.tile_pool(name="res", bufs=1) as res_pool,
            tc.tile_pool(name="outsb", bufs=3) as out_pool,
        ):
            dwt_sb = const.tile([P, NKC * RANK], bf16, name="dwt_sb")
            upw2_sb = const.tile([P, D_OUT], bf16, name="upw2_sb")
            maskt_sb = const.tile([P, NT], bf16, name="maskt_sb")
            nc.sync.dma_start(out=dwt_sb[:], in_=dwt[:, :])

            resT = res_pool.tile([P, NT], bf16, name="resT")

            with tc.tile_pool(name="psum_dn", bufs=1, space="PSUM") as psum_dn_pool:
                # single 4-bank accumulator: partitions 0-63 = even ki
                # chunks, 64-127 = odd ki chunks
                dn = psum_dn_pool.tile([P, NT], f32, name="dn")

                # rotate issuing engines so transfers land on different
                # DGE rings (qSPDynamicHW / qActDynamicHW / SWDGE) and
                # overlap instead of serializing FIFO on one ring
                load_engines = [nc.sync, nc.scalar, nc.gpsimd]
                NQUAD = NKC // 4
                for qr in range(NQUAD):
                    hc = hch_pool.tile([P, 4 * NT], bf16, name="hc")
                    if qr == 0:
                        # split the first load so the first matmul waits on
                        # 512 KiB, not 2 MiB (pipeline fill)
                        for j4 in range(4):
                            load_engines[j4 % 3].dma_start(
                                out=hc[:, j4 * NT:(j4 + 1) * NT],
                                in_=ht4[qr * P:(qr + 1) * P,
                                        j4 * NT:(j4 + 1) * NT],
                            )
                    else:
                        load_engines[qr % 3].dma_start(
                            out=hc[:], in_=ht4[qr * P:(qr + 1) * P, :]
                        )
                    if qr == 1:
                        # needed only from the mask/up phase (~55 us in):
                        # slot the loads behind the first chunk quads
                        nc.sync.dma_start(out=upw2_sb[:], in_=upw2[:, :])
                        nc.sync.dma_start(out=maskt_sb[:], in_=maskt[:, :])
                    for q in range(NQ):
                        for j4 in range(4):
                            ki = 4 * qr + j4
                            j = ki % 2
                            # even ki -> psum partitions 0-63 (array cols
                            # 0-63), odd -> 64-127; the two column-tile
                            # streams run concurrently on the PE
                            nc.tensor.matmul(
                                dn[j * RANK:(j + 1) * RANK,
                                   q * QW:(q + 1) * QW],
                                lhsT=dwt_sb[:, ki * RANK:(ki + 1) * RANK],
                                rhs=hc[:, j4 * NT + q * QW:j4 * NT + (q + 1) * QW],
                                start=(qr == 0 and j4 < 2),
                                stop=(qr == NQUAD - 1 and j4 >= 2),
                                skip_group_check=True,
                            )

                # evict downT psum -> sbuf bf16 fused with the top-k mask;
                # quarter granularity so the up phase starts after q0
                for q in range(NQ):
                    cols = slice(q * QW, (q + 1) * QW)
                    nc.vector.tensor_mul(
                        resT[:, cols], maskt_sb[:, cols], dn[:, cols]
                    )

            # up-proj: outT[oc] = upw2.T @ resT with K=128 (the stacked
            # even/odd partials sum inside the contraction)
            with tc.tile_pool(name="psum_up", bufs=4, space="PSUM") as psum_up_pool:
                for ocp in range(NOCP):
                    osb = out_pool.tile([P, 2 * NT], bf16, name="osb")
                    for oc_in in range(2):
                        oc = 2 * ocp + oc_in
                        for qq in range(2):
                            pu = psum_up_pool.tile([P, 2 * QW], f32, name="pu")
                            for k in range(2):
                                q = 2 * qq + k
                                nc.tensor.matmul(
                                    pu[:, k * QW:(k + 1) * QW],
                                    lhsT=upw2_sb[:, oc * P:(oc + 1) * P],
                                    rhs=resT[:, q * QW:(q + 1) * QW],
                                    start=True,
                                    stop=True,
                                )
                            dst = osb[:, oc_in * NT + qq * 2 * QW:
                                      oc_in * NT + (qq + 1) * 2 * QW]
                            if (oc_in + qq) % 2 == 0:
                                nc.scalar.copy(out=dst, in_=pu[:])
                            else:
                                nc.vector.tensor_copy(out=dst, in_=pu[:])
                    store_engines = [nc.sync, nc.gpsimd]
                    if ocp == NOCP - 1:
                        # split the last store so the final DMA drain on the
                        # critical tail is ~256 KiB, not 1 MiB
                        w = NT // 2
                        for k in range(4):
                            store_engines[k % 2].dma_start(
                                out=outt2[ocp * P:(ocp + 1) * P,
                                          k * w:(k + 1) * w],
                                in_=osb[:, k * w:(k + 1) * w],
                            )
                    else:
                        store_engines[ocp % 2].dma_start(
                            out=outt2[ocp * P:(ocp + 1) * P, :], in_=osb[:]
                        )

    nc.finalize()
    return nc


def _get_program():
    if "nc" not in _CACHE:
        _CACHE["nc"] = _build_program()
    return _CACHE["nc"]


def prepare_in_maps(hidden_states, down_w, up_w, top_k_values, top_k_indices):
    h = np.ascontiguousarray(hidden_states, dtype=np.float32).astype(BF16)
    dw = np.ascontiguousarray(down_w, dtype=np.float32).astype(BF16)
    uw = np.ascontiguousarray(up_w, dtype=np.float32).astype(BF16)
    vals = np.ascontiguousarray(top_k_values, dtype=np.float32)
    idx = np.asarray(top_k_indices).astype(np.int64)

    # dwt[i, ki*64 + r] = dw[r, ki*128 + i]
    dwt = np.ascontiguousarray(
        dw.reshape(RANK, NKC, P).transpose(2, 1, 0).reshape(P, NKC * RANK)
    )
    # up weights transposed and stacked twice: K=128 contraction sums the
    # even-ki (partitions 0-63) and odd-ki (64-127) down partials
    upw2 = np.ascontiguousarray(np.vstack([uw.T, uw.T]))  # [128, 4096]

    rows = np.arange(NT)[:, None]
    in_maps = []
    for c in range(NCORES):
        s = slice(c * NT, (c + 1) * NT)
        # ht4[qr*128+p, j4*2048+n] = h[s][n, (4qr+j4)*128+p]
        ht = h[s].T  # [4096, 2048]
        ht4 = np.ascontiguousarray(
            ht.reshape(NKC // 4, 4, P, NT).transpose(0, 2, 1, 3).reshape(D_IN // 4, 4 * NT)
        )
        m = np.zeros((NT, RANK), dtype=np.float32)
        m[rows, idx[s]] = vals[s]
        mt = m.T.astype(BF16)  # [64, 2048]
        in_maps.append(
            {
                "ht4": ht4,
                "dwt": dwt,
                "upw2": upw2,
                "maskt": np.ascontiguousarray(np.vstack([mt, mt])),  # [128, 2048]
            }
        )
    return in_maps


def gather_output(results):
    # each core returns outt2 [2048, 4096] bf16 with
    # outt2[ocp*128+p, oc_in*2048+n] = outT[(2*ocp+oc_in)*128+p, n];
    # unpack to outT [4096, 2048], transpose, upcast
    outs = []
    for r in results:
        o2 = np.asarray(r["outt2"])
        outT = (
            o2.reshape(NOCP, P, 2, NT)
            .transpose(0, 2, 1, 3)
            .reshape(D_OUT, NT)
        )
        outs.append(outT.T.astype(np.float32))
    return np.concatenate(outs, axis=0)


def kernel(hidden_states, down_w, up_w, top_k_values, top_k_indices, **_kw):
    from concourse.bass_utils import run_bass_kernel_spmd

    nc = _get_program()
    in_maps = prepare_in_maps(
        hidden_states, down_w, up_w, top_k_values, top_k_indices
    )
    res = run_bass_kernel_spmd(nc, in_maps, core_ids=list(range(NCORES)))
    return gather_output(res.results)
